# revision 17
# baseline (speedup 1.0000x reference)
"""ABCNN2 attention pooling kernel for 8 Trainium2 NeuronCores.

Per example (B=128 total, 16 per core):
  a = x1[b,0] (259,512), b = x2[b,0]
  sq[i,j] = ||a_i||^2 + ||b_j||^2 - 2 a_i.b_j      (Gram matmul on PE, K=513)
  A = 1/(1 + sqrt(sq))                             (ACT Sqrt + Copy(+1), DVE divide)
  w_a[k] = sum_i A[i,k]   (ones-matmul on PE)
  w_b[k] = sum_j A[k,j]   (accumulated during the DVE divide)
  y1 = w_a * a ; y2 = w_b * b                      (DVE tensor_scalar)
  w1[j] = sum_{d<4} y1[j+d] ; w2 likewise          (banded const matmul on PE)
Outputs: two (128,1,256,512) f32 tensors.

Sharding: pure data parallelism, batch dim 0 split across 8 cores.
"""

import sys

sys.path.insert(0, "/opt/trn_rl_repo")

import numpy as np
from contextlib import ExitStack

import concourse.bass as bass
import concourse.tile as tile
from concourse import bacc, mybir
from concourse.bass_utils import run_bass_kernel_spmd

F32 = mybir.dt.float32
BF16 = mybir.dt.bfloat16
AF = mybir.ActivationFunctionType
OP = mybir.AluOpType

B, M, D, L, WIN = 128, 259, 512, 256, 4
N_CORES = 8
PER = B // N_CORES  # 16 examples per core
GRP = 4  # examples per input DMA batch

ILENS = [128, 128, 3]
I0S = [0, 128, 256]


def build_body(ctx: ExitStack, tc, o1, o2, x1, x2, n_ex=PER):
    nc = tc.nc
    n_grp = (n_ex + GRP - 1) // GRP

    # ---------------- pools ----------------
    cpool = ctx.enter_context(tc.tile_pool(name="consts", bufs=1))
    ain = ctx.enter_context(tc.tile_pool(name="ain", bufs=2))
    tails = ctx.enter_context(tc.tile_pool(name="tails", bufs=1))
    tp_sb = ctx.enter_context(tc.tile_pool(name="tp_sb", bufs=2))
    apool = ctx.enter_context(tc.tile_pool(name="apool", bufs=2))
    ypool = ctx.enter_context(tc.tile_pool(name="ypool", bufs=2))
    wpool = ctx.enter_context(tc.tile_pool(name="wpool", bufs=2))
    vpool = ctx.enter_context(tc.tile_pool(name="vpool", bufs=3))
    scrpool = ctx.enter_context(tc.tile_pool(name="scr", bufs=2))

    ps_sq = ctx.enter_context(tc.tile_pool(name="ps_sq", bufs=1, space="PSUM"))
    ps_tp = ctx.enter_context(tc.tile_pool(name="ps_tp", bufs=2, space="PSUM"))
    ps_band = ctx.enter_context(tc.tile_pool(name="ps_band", bufs=2, space="PSUM"))
    ps_small = ctx.enter_context(tc.tile_pool(name="ps_small", bufs=1, space="PSUM"))

    # ---------------- constants ----------------
    ones_sq = cpool.tile([128, 128], BF16, tag="ones_sq")
    nc.gpsimd.memset(ones_sq[:], 1.0)
    ident = cpool.tile([128, 128], BF16, tag="ident")
    nc.gpsimd.affine_select(
        ident[:], ones_sq[:], pattern=[[-1, 128]], compare_op=OP.is_equal,
        fill=0.0, base=0, channel_multiplier=1,
    )
    ones_f32 = cpool.tile([128, 128], F32, tag="ones_f32")
    nc.gpsimd.memset(ones_f32[:], 1.0)
    ident_f32 = cpool.tile([128, 128], F32, tag="ident_f32")
    nc.gpsimd.affine_select(
        ident_f32[:], ones_f32[:], pattern=[[-1, 128]], compare_op=OP.is_equal,
        fill=0.0, base=0, channel_multiplier=1,
    )

    # band matrix: B[k, j] = 1 iff 0 <= k - j <= 3  (k partition, j free)
    btmp = cpool.tile([128, 128], BF16, tag="btmp")
    nc.gpsimd.affine_select(
        btmp[:], ones_sq[:], pattern=[[-1, 128]], compare_op=OP.is_ge,
        fill=0.0, base=0, channel_multiplier=1,
    )
    bmain = cpool.tile([128, 128], BF16, tag="bmain")
    nc.gpsimd.affine_select(
        bmain[:], btmp[:], pattern=[[1, 128]], compare_op=OP.is_ge,
        fill=0.0, base=3, channel_multiplier=-1,
    )
    # corner (3, 128): bc128[p, q] = 1 iff q >= 125 + p
    # (band spill rows k=128..130 / 256..258 onto output rows j=125..127 /
    #  253..255; zero elsewhere so the start=True matmul covers the full tile)
    bc128 = cpool.tile([3, 128], BF16, tag="bc128")
    nc.gpsimd.affine_select(
        bc128[:], ones_sq[0:3, :], pattern=[[1, 128]], compare_op=OP.is_ge,
        fill=0.0, base=-125, channel_multiplier=-1,
    )

    ones_row = cpool.tile([1, 128], BF16, tag="ones_row")
    nc.gpsimd.memset(ones_row[:], 1.0)
    ones_col = cpool.tile([128, 1], BF16, tag="ones_col")
    nc.gpsimd.memset(ones_col[:], 1.0)
    ones_a = cpool.tile([128, M], BF16, tag="ones_a")
    nc.gpsimd.memset(ones_a[:], 1.0)

    # ---------------- input DMA (gpsimd SWDGE casts f32->bf16) ----------------
    # tails: rows 256..258 of every example, partitions 0..2
    ta = tails.tile([3, n_ex, D], BF16, tag="ta")
    tb = tails.tile([3, n_ex, D], BF16, tag="tb")
    nc.gpsimd.dma_start(ta[:], x1[:, 0, 256:M, :].rearrange("e p d -> p e d"))
    nc.gpsimd.dma_start(tb[:], x2[:, 0, 256:M, :].rearrange("e p d -> p e d"))

    a_groups, b_groups = [], []
    for g in range(n_grp):
        e0, e1 = g * GRP, min((g + 1) * GRP, n_ex)
        ne = e1 - e0
        ag = ain.tile([128, ne, 2, D], BF16, tag="ag")
        bg = ain.tile([128, ne, 2, D], BF16, tag="bg")
        for c in range(2):
            csl = slice(c * 128, (c + 1) * 128)
            nc.gpsimd.dma_start(
                ag[:, :, c, :], x1[e0:e1, 0, csl, :].rearrange("e p d -> p e d")
            )
            nc.gpsimd.dma_start(
                bg[:, :, c, :], x2[e0:e1, 0, csl, :].rearrange("e p d -> p e d")
            )
        a_groups.append(ag)
        b_groups.append(bg)

    # ---------------- per-example pipeline ----------------
    for e in range(n_ex):
        g, s = e // GRP, e % GRP
        ag, bg = a_groups[g], b_groups[g]
        chunks_a = [ag[:, s, 0, :], ag[:, s, 1, :], ta[0:3, e, :]]
        chunks_b = [bg[:, s, 0, :], bg[:, s, 1, :], tb[0:3, e, :]]

        # --- norms: na/nb via squared-sum (POOL stt with accumulate) ---
        na = vpool.tile([128, 3], F32, tag="na")
        nb = vpool.tile([128, 3], F32, tag="nb")
        for c in range(3):
            pl = ILENS[c]
            scr = scrpool.tile([128, D], BF16, tag="scr")
            nc.vector.scalar_tensor_tensor(
                scr[0:pl, :], chunks_a[c], 1.0, chunks_a[c],
                op0=OP.mult, op1=OP.mult, accum_out=na[0:pl, c : c + 1],
            )
            scr2 = scrpool.tile([128, D], BF16, tag="scr")
            nc.vector.scalar_tensor_tensor(
                scr2[0:pl, :], chunks_b[c], 1.0, chunks_b[c],
                op0=OP.mult, op1=OP.mult, accum_out=nb[0:pl, c : c + 1],
            )

        # --- nb as a row (1, M): PE mini-transposes ---
        nbrow_ps = ps_small.tile([1, M], F32, tag="sm")
        for c in range(3):
            pl, i0 = ILENS[c], I0S[c]
            nc.tensor.transpose(
                nbrow_ps[0:1, i0 : i0 + pl], nb[0:pl, c : c + 1], ident_f32[0:pl, 0:pl]
            )
        nbrow = vpool.tile([1, M], BF16, tag="nbrow")
        nc.scalar.mul(nbrow[:], nbrow_ps[:], -0.5)

        # --- transposes: aTm2 = -2*a^T, bT = b^T  (d-major, bf16) ---
        aTm2 = tp_sb.tile([128, 4, M], BF16, tag="aTm2")
        bT = tp_sb.tile([128, 4, M], BF16, tag="bT")
        for (dst, chunks, idm) in ((aTm2, chunks_a, ident), (bT, chunks_b, ident)):
            for k in range(4):
                ksl = slice(k * 128, (k + 1) * 128)
                tp = ps_tp.tile([128, M], BF16, tag="tp")
                nc.tensor.transpose(tp[:, 0:128], chunks[0][:, ksl], idm[:])
                nc.tensor.transpose(tp[:, 128:256], chunks[1][:, ksl], idm[:])
                nc.tensor.transpose(tp[:, 256:M], chunks[2][:, ksl], idm[0:3, 0:3])
                nc.scalar.copy(dst[:, k, :], tp[:])

        # --- Gram: sq = -2 a.b + nb_j (+ na_i via Sqrt bias below) ---
        sq = ps_sq.tile([128, 3, 512], F32, tag="sq")
        for c in range(3):
            pl, i0 = ILENS[c], I0S[c]
            sqc = sq[0:pl, c, 0:M]
            for k in range(4):
                nc.tensor.matmul(
                    sqc, aTm2[:, k, i0 : i0 + pl], bT[:, k, :],
                    start=(k == 0), stop=False,
                )
            nc.tensor.matmul(
                sqc, ones_row[0:1, 0:pl], nbrow[0:1, :], start=False, stop=True
            )

        # --- A = 1/(1+sqrt(sq)) via Ln/Exp chain, all on ACT (one LUT table):
        #     u = ln(na - 2*psum) ; d = exp(u/2) = sqrt(sq) ;
        #     t = ln(1 + d) ; A = exp(-t), accum -> w_b
        for c in range(3):
            pl = ILENS[c]
            nc.scalar.activation(
                sq[0:pl, c, 0:M], sq[0:pl, c, 0:M], AF.Ln,
                bias=na[0:pl, c : c + 1], scale=-2.0,
            )
        nc.scalar.activation(sq[:, 0:2, 0:M], sq[:, 0:2, 0:M], AF.Exp, scale=0.5)
        nc.scalar.activation(sq[0:3, 2, 0:M], sq[0:3, 2, 0:M], AF.Exp, scale=0.5)
        nc.scalar.activation(sq[:, 0:2, 0:M], sq[:, 0:2, 0:M], AF.Ln, bias=1.0)
        nc.scalar.activation(sq[0:3, 2, 0:M], sq[0:3, 2, 0:M], AF.Ln, bias=1.0)
        amat = apool.tile([128, 3, M], BF16, tag="amat")
        wb = vpool.tile([128, 3], F32, tag="wb")
        for c in range(3):
            pl = ILENS[c]
            nc.scalar.activation(
                amat[0:pl, c, :], sq[0:pl, c, 0:M], AF.Exp, scale=-1.0,
                accum_out=wb[0:pl, c : c + 1],
            )

        # --- w_a = col-sums of A: ones-matmul -> row -> transpose to col ---
        warow_ps = ps_small.tile([1, M], F32, tag="sm")
        for c in range(3):
            pl = ILENS[c]
            nc.tensor.matmul(
                warow_ps[0:1, :], ones_col[0:pl, 0:1], amat[0:pl, c, :],
                start=(c == 0), stop=(c == 2),
            )
        warow = vpool.tile([1, M], F32, tag="warow")
        nc.scalar.copy(warow[:], warow_ps[:])
        wacol_ps = ps_small.tile([128, 3], F32, tag="sm")
        for c in range(3):
            pl, i0 = ILENS[c], I0S[c]
            nc.tensor.transpose(
                wacol_ps[0:pl, c : c + 1], warow[0:1, i0 : i0 + pl],
                ident_f32[0:1, 0:1],
            )
        wa = vpool.tile([128, 3], F32, tag="wa")
        nc.vector.tensor_copy(wa[:, 0:2], wacol_ps[:, 0:2])
        nc.vector.tensor_copy(wa[0:3, 2:3], wacol_ps[0:3, 2:3])

        # --- y1 = w_a * a, y2 = w_b * b (bf16) ---
        y1 = ypool.tile([128, 2, D], BF16, tag="y1")
        y2 = ypool.tile([128, 2, D], BF16, tag="y2")
        y1t = ypool.tile([3, D], BF16, tag="y1t")
        y2t = ypool.tile([3, D], BF16, tag="y2t")
        for c in range(2):
            nc.vector.tensor_scalar_mul(y1[:, c, :], chunks_a[c], wa[:, c : c + 1])
            nc.vector.tensor_scalar_mul(y2[:, c, :], chunks_b[c], wb[:, c : c + 1])
        nc.vector.tensor_scalar_mul(y1t[:], chunks_a[2], wa[0:3, 2:3])
        nc.vector.tensor_scalar_mul(y2t[:], chunks_b[2], wb[0:3, 2:3])

        # --- banded window-sum matmuls; copy psum->sbuf; DMA out ---
        for (y, yt, wsb_pool_tag, out_ap) in (
            (y1, y1t, "w1sb", o1), (y2, y2t, "w2sb", o2)
        ):
            wsb = wpool.tile([128, 2, D], F32, tag=wsb_pool_tag)
            pb0 = ps_band.tile([128, D], F32, tag="pb")
            nc.tensor.matmul(
                pb0[:], bc128[:], y[0:3, 1, :], start=True, stop=False,
                skip_group_check=True,
            )
            nc.tensor.matmul(
                pb0[:], bmain[:], y[:, 0, :], start=False, stop=True,
                skip_group_check=True,
            )
            pb1 = ps_band.tile([128, D], F32, tag="pb")
            nc.tensor.matmul(
                pb1[:], bc128[:], yt[0:3, :], start=True, stop=False,
                skip_group_check=True,
            )
            nc.tensor.matmul(
                pb1[:], bmain[:], y[:, 1, :], start=False, stop=True,
                skip_group_check=True,
            )
            nc.scalar.copy(wsb[:, 0, :], pb0[:])
            nc.vector.tensor_copy(wsb[:, 1, :], pb1[:])
            nc.sync.dma_start(
                out_ap[e, 0, :, :].rearrange("(c p) d -> p c d", p=128), wsb[:]
            )


def build_nc(n_ex=PER, n_devices=N_CORES):
    nc = bacc.Bacc(
        "TRN2", target_bir_lowering=False, debug=False,
        enable_asserts=False, num_devices=n_devices,
    )
    x1 = nc.dram_tensor("x1", [n_ex, 1, M, D], F32, kind="ExternalInput").ap()
    x2 = nc.dram_tensor("x2", [n_ex, 1, M, D], F32, kind="ExternalInput").ap()
    o1 = nc.dram_tensor("o1", [n_ex, 1, L, D], F32, kind="ExternalOutput").ap()
    o2 = nc.dram_tensor("o2", [n_ex, 1, L, D], F32, kind="ExternalOutput").ap()
    with tile.TileContext(nc) as tc:
        with ExitStack() as ctx:
            build_body(ctx, tc, o1, o2, x1, x2, n_ex=n_ex)
    nc.compile()
    return nc


_NC_CACHE = {}


def kernel(x1: np.ndarray, x2: np.ndarray):
    x1 = np.ascontiguousarray(x1, dtype=np.float32)
    x2 = np.ascontiguousarray(x2, dtype=np.float32)
    assert x1.shape == (B, 1, M, D) and x2.shape == (B, 1, M, D)

    if "nc" not in _NC_CACHE:
        _NC_CACHE["nc"] = build_nc()
    nc = _NC_CACHE["nc"]

    in_maps = []
    for c in range(N_CORES):
        sl = slice(c * PER, (c + 1) * PER)
        in_maps.append({"x1": x1[sl], "x2": x2[sl]})

    res = run_bass_kernel_spmd(nc, in_maps, core_ids=list(range(N_CORES)))
    outs = res.results
    w1 = np.concatenate([outs[c]["o1"] for c in range(N_CORES)], axis=0)
    w2 = np.concatenate([outs[c]["o2"] for c in range(N_CORES)], axis=0)
    return w1, w2
